# revision 14
# baseline (speedup 1.0000x reference)
"""2-layer GCN (PyG GCNConv semantics) on 8 Trainium2 NeuronCores.

Strategy (self-contained; shapes hardcoded for the nn_GCNEncoder problem):
  - Nodes relabeled (degree-balanced) and partitioned across 8 cores
    (12544 padded nodes each = 98 blocks of 128).
  - Layer algebra:
      table1 = dinv * (x @ W1)                  [per-shard matmul + AllGather]
      s1     = segsum(table1[src]) + self       -> h1s = dinv^2 * relu(s1)
      table2 = h1s @ W2  (W2 folded into the table -> 32-wide L2 messages)
      out    = dinv * (segsum(table2[src]) + self) + b2
  - Edge aggregation: edges sorted by (core, gather-group of 16 dst blocks,
    source region, dst slot). Chunks of 128 edges are aligned to slot-
    stratified windows (cross-core max count <= 128 per window, width <= 32
    slots). Each chunk: SWDGE row-gather (dense, no skips; pads duplicate a
    real row) then ONE TensorE matmul: stationary = gathered messages
    [128 edges, F], moving = narrow one-hot [128, w<=32] built on DVE by
    is_equal(colid, dst-lo), accumulated into a transposed PSUM bank
    [F, 512 slots] per 4-block group (started by a self-loop matmul
    W.T @ featT which also initializes the zero region).
  - The transposed PSUM layout makes the W2 fold and the L2 self-init plain
    matmuls (no transposes anywhere); final output is written transposed
    [32, nodes] and re-transposed on host.
"""
import numpy as np
from contextlib import ExitStack

N_REAL = 100000
N_PAD = 100352            # 8 * 98 * 128
NCORES = 8
NSHARD = N_PAD // NCORES  # 12544
NBLOCKS = NSHARD // 128   # 98
P = 128
F1 = 64                   # hidden width (W1 out)
F2 = 32                   # output width (W2 out)
IN_C = 128
GG_SIZES = [6, 12, 16, 16, 16, 16, 10, 4, 2]   # blocks per gather group
assert sum(GG_SIZES) == NBLOCKS
NGG = len(GG_SIZES)
GG_B0 = np.concatenate([[0], np.cumsum(GG_SIZES)]).astype(np.int64)  # block starts
G_GG = max(GG_SIZES)
GGW = G_GG * P            # max slots per gather group (2048)
NGRP = 4                  # source regions per layer
NCOL = 32                 # one-hot window width (host schedule asserts)
CB = 64                   # chunks per DVE one-hot build batch
NPAIR = N_PAD // 2        # 50176 L1 pair rows
SEGP = NPAIR // 2         # 25088 rows per L1 segment
NQUAD = N_PAD // 4        # 25088 L2 quad rows


def _bf16(a):
    import ml_dtypes
    return np.asarray(a, np.float32).astype(ml_dtypes.bfloat16)


def _balanced_perm(deg):
    """Assign nodes to 784 blocks of 128 balancing per-block degree sums.
    Returns perm: orig_id -> new_id (new_id = block*128 + slot)."""
    import heapq
    nblocks_g = N_PAD // P
    order = np.argsort(-deg, kind="stable")
    blocks = np.empty(N_REAL, np.int64)
    heap = [(0, b) for b in range(nblocks_g)]
    heapq.heapify(heap)
    fill = np.zeros(nblocks_g, np.int64)
    deg_sorted = deg[order]
    for i in range(N_REAL):
        load, b = heapq.heappop(heap)
        blocks[order[i]] = b
        fill[b] += 1
        if fill[b] < P:
            heapq.heappush(heap, (load + int(deg_sorted[i]), b))
    o2 = np.argsort(blocks, kind="stable")
    counts = np.bincount(blocks, minlength=nblocks_g)
    starts = np.concatenate([[0], np.cumsum(counts)[:-1]])
    slots = np.arange(N_REAL) - starts[blocks[o2]]
    new_ids = blocks[o2] * P + slots
    perm_real = np.empty(N_REAL, np.int64)
    perm_real[o2] = new_ids
    used = np.zeros(N_PAD, bool)
    used[perm_real] = True
    return np.concatenate([perm_real, np.flatnonzero(~used)])


def _build_schedule(new_src, new_dst, grp, rowidx):
    """Exact-128-edge chunks with cross-core union windows; -1 tail padding
    (trimmed free by the SWDGE ucode). Windows are split at PSUM bank
    boundaries at emission time."""
    core = new_dst // NSHARD
    block = (new_dst % NSHARD) // P
    gg = np.searchsorted(GG_B0, block, side="right") - 1
    slot = (new_dst % NSHARD) - GG_B0[gg] * P

    ncalls = NGG * NGRP
    call_of_edge = gg * NGRP + grp
    okey = (core * ncalls + call_of_edge) * (GGW + P) + slot
    eorder = np.argsort(okey, kind="stable")
    s_idx = rowidx[eorder].astype(np.int64)
    s_slot = slot[eorder]
    ckey = (core * ncalls + call_of_edge)[eorder]
    cnt = np.bincount(ckey, minlength=NCORES * ncalls).reshape(NCORES, ncalls)
    estart = np.concatenate([[0], np.cumsum(cnt.reshape(-1))[:-1]]).reshape(
        NCORES, ncalls)

    k_call = np.maximum(1, np.ceil(cnt.max(axis=0) / P).astype(np.int64))
    kmax = int(k_call.max())
    caps = k_call * P
    call_base = np.concatenate([[0], np.cumsum(caps)[:-1]])
    nidx = int(caps.sum())
    nchunks = int(k_call.sum())
    chunk_base = np.concatenate([[0], np.cumsum(k_call)[:-1]])

    idx_arr = np.full((NCORES, nidx), -1, np.int16)
    dstv = np.full((NCORES, P, nchunks), -7.0, np.float32)
    win_lo = np.full((ncalls, kmax), 10 ** 9, np.int64)
    win_hi = np.full((ncalls, kmax), -1, np.int64)

    for ci in range(ncalls):
        for c in range(NCORES):
            n = cnt[c, ci]
            b = estart[c, ci]
            if n == 0:
                idx_arr[c, call_base[ci]] = 0
                continue
            sl = s_slot[b:b + n]
            ix = s_idx[b:b + n]
            pos = np.arange(n)
            idx_arr[c, call_base[ci]:call_base[ci] + n] = ix
            wj = pos // P
            np.minimum.at(win_lo[ci], wj, sl)
            np.maximum.at(win_hi[ci], wj, sl)
            dstv[c, pos % P, chunk_base[ci] + wj] = sl

    # relative dst values + window metadata (split at 512-slot banks)
    ncol = 32
    calls = []
    for ci in range(ncalls):
        k = int(k_call[ci])
        meta = []
        for j in range(k):
            lo = int(win_lo[ci, j])
            hi = int(win_hi[ci, j])
            if hi < 0:
                lo, hi = 0, 0
            span = hi - lo + 1
            ncol = max(ncol, span)
            pieces = []
            p0 = lo
            while p0 <= hi:
                pe = min(hi, ((p0 // 512) + 1) * 512 - 1)
                pieces.append([p0, pe - p0 + 1, p0 // 512, False])
                p0 = pe + 1
            meta.append((lo, pieces))
        calls.append({"gg": ci // NGRP, "grp": ci % NGRP, "k": k,
                      "base": int(call_base[ci]),
                      "cbase": int(chunk_base[ci]), "windows": meta})
    ncol = int(np.ceil(ncol / 32) * 32)
    assert ncol <= 256, f"window span {ncol} too wide"

    # dstv relative to window lo
    for ci in range(ncalls):
        k = int(k_call[ci])
        lows = np.array([calls[ci]["windows"][j][0] for j in range(k)],
                        np.float64)
        cb = calls[ci]["cbase"]
        dv = dstv[:, :, cb:cb + k]
        mask = dv >= 0
        dv[mask] = (dv - lows[None, None, :])[mask]

    # stop flag: last piece (consumption order) touching each (gg, pg)
    for ggi in range(NGG):
        last = {}
        for gi in range(NGRP):
            ci = ggi * NGRP + gi
            for j, (lo, pieces) in enumerate(calls[ci]["windows"]):
                for pi, pc in enumerate(pieces):
                    last[pc[2]] = (ci, j, pi)
        for (ci, j, pi) in last.values():
            calls[ci]["windows"][j][1][pi][3] = True

    gg_off, gg_len, gg_cb = [], [], []
    for ggi in range(NGG):
        o = int(call_base[ggi * NGRP])
        e = nidx if ggi == NGG - 1 else int(call_base[(ggi + 1) * NGRP])
        gg_off.append(o)
        gg_len.append(e - o)
        gg_cb.append(int(chunk_base[ggi * NGRP]))
    gg_cb.append(nchunks)

    return {"calls": calls, "idx_arr": idx_arr, "dstv": dstv,
            "gg_off": gg_off, "gg_len": gg_len, "gg_cb": gg_cb,
            "nchunks": nchunks, "nidx": nidx, "kmax": kmax, "ncol": ncol,
            "cnt": cnt.astype(np.int32)}


def _wrap_idx(idx_flat):
    """SWDGE wrapped int16 idx layout: [16, n/16] pattern tiled to 128."""
    n = idx_flat.shape[0]
    cols = idx_flat.astype(np.int16).reshape(n // 16, 16).T
    return np.tile(cols, (8, 1))


def _make_runner(nc, n_cores):
    """Build the bass2jax PJRT executable once; return a callable."""
    import time
    import jax
    from jax.sharding import Mesh, PartitionSpec, NamedSharding
    from jax.experimental.shard_map import shard_map
    import concourse.mybir as mybir
    from concourse.bass2jax import (_bass_exec_p, install_neuronx_cc_hook,
                                    partition_id_tensor)

    install_neuronx_cc_hook()
    partition_name = nc.partition_id_tensor.name if nc.partition_id_tensor else None

    in_names, out_names, out_avals, zero_outs = [], [], [], []
    for alloc in nc.m.functions[0].allocations:
        if not isinstance(alloc, mybir.MemoryLocationSet):
            continue
        name = alloc.memorylocations[0].name
        if alloc.kind == "ExternalInput":
            if name != partition_name:
                in_names.append(name)
        elif alloc.kind == "ExternalOutput":
            out_names.append(name)
            shape = tuple(alloc.tensor_shape)
            dtype = mybir.dt.np(alloc.dtype)
            out_avals.append(jax.core.ShapedArray(shape, dtype))
            zero_outs.append(np.zeros(shape, dtype))
    n_params = len(in_names)
    n_outs = len(out_avals)
    all_in_names = list(in_names) + list(out_names)
    if partition_name is not None:
        all_in_names.append(partition_name)

    def _body(*args):
        operands = list(args)
        if partition_name is not None:
            operands.append(partition_id_tensor())
        outs = _bass_exec_p.bind(
            *operands,
            out_avals=tuple(out_avals),
            in_names=tuple(all_in_names),
            out_names=tuple(out_names),
            lowering_input_output_aliases=(),
            sim_require_finite=False,
            sim_require_nnan=False,
            nc=nc,
        )
        return tuple(outs)

    devices = jax.devices()[:n_cores]
    mesh = Mesh(np.asarray(devices), ("core",))
    in_specs = (PartitionSpec("core"),) * (n_params + n_outs)
    out_specs = (PartitionSpec("core"),) * len(out_names)
    sharded = jax.jit(
        shard_map(_body, mesh=mesh, in_specs=in_specs, out_specs=out_specs,
                  check_rep=False),
        keep_unused=True,
    )

    def run(in_maps, n_iters=0):
        shard = NamedSharding(mesh, PartitionSpec("core"))
        per_core = [[np.asarray(m[name]) for name in in_names] for m in in_maps]
        concat_in = [
            np.concatenate([per_core[c][i] for c in range(n_cores)], axis=0)
            for i in range(n_params)
        ]
        concat_zeros = [
            np.zeros((n_cores * z.shape[0], *z.shape[1:]), z.dtype) for z in zero_outs
        ]
        args = [jax.device_put(a, shard) for a in concat_in + concat_zeros]
        out = sharded(*args)
        jax.block_until_ready(out)
        times = []
        for _ in range(n_iters):
            t0 = time.perf_counter()
            out = sharded(*args)
            jax.block_until_ready(out)
            times.append(time.perf_counter() - t0)
        results = [
            {name: np.asarray(out[i]).reshape(n_cores, *out_avals[i].shape)[c]
             for i, name in enumerate(out_names)}
            for c in range(n_cores)
        ]
        return results, times

    return run


def _compile_and_make_runner(sch1, sch2, ncol, with_b2):
    import concourse.bass as bass
    import concourse.bacc as bacc
    import concourse.mybir as mybir
    import concourse.tile as tile
    from concourse.bass import exact_div

    dt = mybir.dt

    def dma_gather(gp, out_ap, in_ap, idxs_ap, num_idxs, nreg, elem_size,
                   elem_step, q):
        stride_bytes_256 = exact_div(elem_step * dt.size(in_ap.dtype), 256)
        _in_ap = gp.lower_ap_dma(in_ap, for_custom_bir_dma=True)
        return gp.add_instruction(
            mybir.InstDMAGatherAnt(
                name=gp.bass.get_next_instruction_name(),
                ins=[*_in_ap, gp.lower_ap(idxs_ap),
                     gp.lower_val_access(nreg)],
                outs=[gp.lower_ap(out_ap)],
                transpose=False, num_idxs=num_idxs, elem_size=elem_size,
                stride_bytes_256=stride_bytes_256, gen_mode=0,
                single_packet=False, queue_num=q))

    CBL = max(4, 2048 // ncol)
    nc = bacc.Bacc("TRN2", target_bir_lowering=False, debug=False,
                   num_devices=NCORES, num_swdge_queues=4,
                   dynamic_dma_scratch_size=32768)

    xT = nc.dram_tensor("xT", [P, NSHARD], dt.bfloat16, kind="ExternalInput").ap()
    w1 = nc.dram_tensor("w1", [P, F1], dt.bfloat16, kind="ExternalInput").ap()
    w2 = nc.dram_tensor("w2", [F1, F2], dt.bfloat16, kind="ExternalInput").ap()
    colid = nc.dram_tensor("colid", [P, ncol], dt.bfloat16, kind="ExternalInput").ap()
    cnt1 = nc.dram_tensor("cnt1", [1, NGG * NGRP], dt.int32, kind="ExternalInput").ap()
    cnt2 = nc.dram_tensor("cnt2", [1, NGG * NGRP], dt.int32, kind="ExternalInput").ap()
    dinv2r = nc.dram_tensor("dinv2r", [F1, NSHARD], dt.bfloat16, kind="ExternalInput").ap()
    dinvr = nc.dram_tensor("dinvr", [F2, NSHARD], dt.float32, kind="ExternalInput").ap()
    idx1 = nc.dram_tensor("idx1", [P, sch1["nidx"] // 16], dt.int16, kind="ExternalInput").ap()
    idx2 = nc.dram_tensor("idx2", [P, sch2["nidx"] // 16], dt.int16, kind="ExternalInput").ap()
    dstv1 = nc.dram_tensor("dstv1", [P, sch1["nchunks"]], dt.bfloat16, kind="ExternalInput").ap()
    dstv2 = nc.dram_tensor("dstv2", [P, sch2["nchunks"]], dt.bfloat16, kind="ExternalInput").ap()
    if with_b2:
        b2c = nc.dram_tensor("b2c", [F2, 1], dt.float32, kind="ExternalInput").ap()
    outT = nc.dram_tensor("outT", [F2, NSHARD], dt.float32, kind="ExternalOutput").ap()

    t1_shard = nc.dram_tensor("t1_shard", [NPAIR // NCORES, P], dt.bfloat16).ap()
    g1_full = nc.dram_tensor("g1_full", [NPAIR, P], dt.bfloat16, addr_space="Shared").ap()
    t2_shard = nc.dram_tensor("t2_shard", [NQUAD // NCORES, P], dt.bfloat16).ap()
    g2_full = nc.dram_tensor("g2_full", [NQUAD, P], dt.bfloat16, addr_space="Shared").ap()

    ggs = [GG_SIZES[g] * P for g in range(NGG)]      # slots per GG
    ggc0 = [int(GG_B0[g]) * P for g in range(NGG)]   # slot offset of GG
    max_ilen1 = max(sch1["gg_len"]) // 16
    max_ilen2 = max(sch2["gg_len"]) // 16
    max_ch1 = max(sch1["gg_cb"][g + 1] - sch1["gg_cb"][g] for g in range(NGG))
    max_ch2 = max(sch2["gg_cb"][g + 1] - sch2["gg_cb"][g] for g in range(NGG))

    with tile.TileContext(nc) as tc, ExitStack() as ctx:
        sb = ctx.enter_context(tc.tile_pool(name="sb", bufs=1))
        sbx = ctx.enter_context(tc.tile_pool(name="sbx", bufs=2))
        sbg1 = ctx.enter_context(tc.tile_pool(name="sbg1", bufs=7))
        sbg2 = ctx.enter_context(tc.tile_pool(name="sbg2", bufs=7))
        sbi = ctx.enter_context(tc.tile_pool(name="sbi", bufs=2))
        sbs = ctx.enter_context(tc.tile_pool(name="sbs", bufs=4))
        sbe = ctx.enter_context(tc.tile_pool(name="sbe", bufs=2))
        sbst = ctx.enter_context(tc.tile_pool(name="sbst", bufs=2))
        ps_agg = ctx.enter_context(tc.tile_pool(name="psagg", bufs=6,
                                                space="PSUM"))
        ps_s = ctx.enter_context(tc.tile_pool(name="pss", bufs=2, space="PSUM"))

        # ---- resident tiles ----
        w1_sb = sb.tile([P, F1], dt.bfloat16)
        w2_sb = sb.tile([F1, F2], dt.bfloat16)
        colid_sb = sb.tile([P, ncol], dt.bfloat16)
        cnt1_sb = sb.tile([1, NGG * NGRP], dt.int32)
        cnt2_sb = sb.tile([1, NGG * NGRP], dt.int32)
        h1Ts = sb.tile([F1, NSHARD], dt.bfloat16)
        nc.scalar.dma_start(w1_sb[:], w1)
        nc.scalar.dma_start(w2_sb[:], w2)
        nc.scalar.dma_start(colid_sb[:], colid)
        nc.scalar.dma_start(cnt1_sb[:], cnt1)
        nc.scalar.dma_start(cnt2_sb[:], cnt2)
        cnt_reg = nc.gpsimd.alloc_register("gather_cnt")
        if with_b2:
            b2_sb = sb.tile([F2, 1], dt.float32)
            nc.scalar.dma_start(b2_sb[:], b2c)

        for _zi in range(7):
            zt1 = sbg1.tile([P, sch1["kmax"] * F1], dt.bfloat16, tag="gb1",
                            name="gb1")
            nc.vector.memset(zt1[:], 0)
            zt2 = sbg2.tile([P, sch2["kmax"] * F2], dt.bfloat16, tag="gb2",
                            name="gb2")
            nc.vector.memset(zt2[:], 0)

        # ---- phase 1: t1 rows = (dinv*x) @ W1, pair-packed to t1_shard ----
        # node (b, s) -> pair row b*64 + s//2, col (s%2)*64
        t1v = t1_shard.rearrange("(b s2) (two f) -> (s2 two) b f", s2=64, f=F1)
        for ggi in range(NGG):
            nb = ggs[ggi] // P
            c0 = ggc0[ggi]
            xgg = sbx.tile([P, GGW], dt.bfloat16, tag="xgg")
            nc.sync.dma_start(xgg[:, :ggs[ggi]], xT[:, c0:c0 + ggs[ggi]])
            stg = sbst.tile([P, G_GG * F1], dt.bfloat16, tag="p1st")
            for g8 in range(0, nb, 8):
                n8 = min(8, nb - g8)
                h_ps = ps_agg.tile([P, 512], dt.float32, tag="agg",
                                   name="aggps")
                for b in range(g8, g8 + n8):
                    bo = (b - g8) * F1
                    nc.tensor.matmul(h_ps[:, bo:bo + F1],
                                     lhsT=xgg[:, b * P:(b + 1) * P],
                                     rhs=w1_sb[:], start=(b == g8),
                                     stop=(b == g8 + n8 - 1))
                nc.scalar.copy(stg[:, g8 * F1:(g8 + n8) * F1],
                               h_ps[:, :n8 * F1])
            nc.scalar.dma_start(
                t1v[:, int(GG_B0[ggi]):int(GG_B0[ggi]) + nb],
                stg[:, :nb * F1].rearrange("p (b f) -> p b f", f=F1))

        nc.gpsimd.collective_compute(
            "AllGather", mybir.AluOpType.bypass,
            replica_groups=[list(range(NCORES))],
            ins=[t1_shard], outs=[g1_full])

        def edge_phase(layer, sch, idx_dram, dstv_dram, F, gpool, max_ilen,
                       max_ch, pre_gg, self_init, post_pg):
            calls = sch["calls"]
            st = {"tile": None, "b0": -1}

            for ggi in range(NGG):
                npg = (ggs[ggi] + 511) // 512
                aux = pre_gg(ggi)
                islab = sbi.tile([P, max_ilen], dt.int16, tag="isl")
                o, ln = sch["gg_off"][ggi], sch["gg_len"][ggi]
                nc.sync.dma_start(islab[:, :ln // 16],
                                    idx_dram[:, o // 16:(o + ln) // 16])
                cb0, cb1 = sch["gg_cb"][ggi], sch["gg_cb"][ggi + 1]
                dvt = sbi.tile([P, max_ch], dt.bfloat16, tag="dvt")
                nc.sync.dma_start(dvt[:, :cb1 - cb0],
                                    dstv_dram[:, cb0:cb1])
                st["tile"], st["b0"] = None, -1

                def get_s2(lj):
                    b0 = (lj // CBL) * CBL
                    if st["b0"] != b0:
                        cbn = min(CBL, (cb1 - cb0) - b0)
                        s2 = sbs.tile([P, CBL * ncol], dt.bfloat16, tag="s2")
                        nc.vector.tensor_tensor(
                            out=s2[:].rearrange("p (c j) -> p c j", c=CBL)[:, :cbn, :],
                            in0=colid_sb[:, None, :].to_broadcast((P, cbn, ncol)),
                            in1=dvt[:, b0:b0 + cbn, None].to_broadcast(
                                (P, cbn, ncol)),
                            op=mybir.AluOpType.is_equal)
                        st["tile"], st["b0"] = s2, b0
                    return st["tile"], lj - b0

                gbufs = []
                for gi in range(NGRP):
                    ci = ggi * NGRP + gi
                    k = calls[ci]["k"]
                    n = k * P
                    gb = gpool.tile([P, sch["kmax"] * F], dt.bfloat16,
                                    tag=f"gb{layer}", name=f"gb{layer}")
                    nc.gpsimd.reg_load(cnt_reg, (cnt1_sb if layer == 1
                                                 else cnt2_sb)[0:1, ci:ci + 1])
                    co = calls[ci]["base"] - o
                    if layer == 1:
                        seg, par = gi // 2, gi % 2
                        src_ap = g1_full[seg * SEGP:(seg + 1) * SEGP,
                                         par * F1:(par + 1) * F1]
                    else:
                        src_ap = g2_full[:, gi * F2:(gi + 1) * F2]
                    dma_gather(
                        nc.gpsimd,
                        gb[:, :k * F].rearrange("p (c f) -> p c f", f=F),
                        src_ap,
                        islab[:, co // 16:(co + n) // 16],
                        num_idxs=n, nreg=cnt_reg, elem_size=F, elem_step=P,
                        q=gi)
                    gbufs.append(gb)

                aggs = []
                for pg in range(npg):
                    w = min(512, ggs[ggi] - pg * 512)
                    agg = ps_agg.tile([F1, 512], dt.float32, tag="agg",
                                      name="aggps")
                    self_init(ggi, pg, w, agg, aux)
                    aggs.append(agg)

                for gi in range(NGRP):
                    ci = ggi * NGRP + gi
                    cbase = calls[ci]["cbase"] - cb0
                    for j, (lo, pieces) in enumerate(calls[ci]["windows"]):
                        s2, coff = get_s2(cbase + j)
                        for (plo, pw, pg, stop) in pieces:
                            nc.tensor.matmul(
                                aggs[pg][:F if layer == 2 else F1,
                                         plo - pg * 512:plo - pg * 512 + pw],
                                lhsT=gbufs[gi][:, j * F:(j + 1) * F],
                                rhs=s2[:, coff * ncol + (plo - lo):
                                       coff * ncol + (plo - lo) + pw],
                                start=False, stop=stop)

                for pg in range(npg):
                    w = min(512, ggs[ggi] - pg * 512)
                    post_pg(ggi, pg, ggc0[ggi] + pg * 512, w, aggs[pg], aux)

        # ---- layer 1 ----
        def pre_gg1(ggi):
            c0, w = ggc0[ggi], ggs[ggi]
            xgg = sbx.tile([P, GGW], dt.bfloat16, tag="xgg", name="xgg")
            nc.sync.dma_start(xgg[:, :w], xT[:, c0:c0 + w])
            d2 = sbx.tile([F1, GGW], dt.bfloat16, tag="d2gg", name="d2gg")
            nc.sync.dma_start(d2[:, :w], dinv2r[:, c0:c0 + w])
            t2st = sbst.tile([P, G_GG * F2], dt.bfloat16, tag="t2st",
                             name="t2st")
            return {"xgg": xgg, "d2": d2, "t2st": t2st}

        def self_init1(ggi, pg, w, agg, aux):
            nc.tensor.matmul(agg[:, :w], lhsT=w1_sb[:],
                             rhs=aux["xgg"][:, pg * 512:pg * 512 + w],
                             start=True, stop=False)

        def post1(ggi, pg, lo, w, agg, aux):
            tr = sbe.tile([F1, 512], dt.bfloat16, tag="trelu")
            nc.scalar.activation(tr[:, :w], agg[:, :w],
                                 mybir.ActivationFunctionType.Relu)
            nc.vector.tensor_tensor(
                h1Ts[:, lo:lo + w], tr[:, :w],
                aux["d2"][:, pg * 512:pg * 512 + w], mybir.AluOpType.mult)
            for bl in range(w // P):
                b = lo // P + bl
                t2_ps = ps_s.tile([P, F2], dt.float32, tag="sps")
                nc.tensor.matmul(t2_ps[:], lhsT=h1Ts[:, b * P:(b + 1) * P],
                                 rhs=w2_sb[:], start=True, stop=True)
                nc.scalar.copy(
                    aux["t2st"][:, (pg * 4 + bl) * F2:(pg * 4 + bl + 1) * F2],
                    t2_ps[:])
            if lo + w == (ggc0[ggi] + ggs[ggi]):
                # quad-packed write: node (b,s) -> quad row b*32+s//4, col (s%4)*32
                nb = ggs[ggi] // P
                t2v = t2_shard.rearrange("(b s4) (q f) -> (s4 q) b f",
                                         s4=32, f=F2)
                nc.scalar.dma_start(
                    t2v[:, int(GG_B0[ggi]):int(GG_B0[ggi]) + nb],
                    aux["t2st"][:, :nb * F2].rearrange("p (b f) -> p b f", f=F2))

        edge_phase(1, sch1, idx1, dstv1, F1, sbg1, max_ilen1, max_ch1,
                   pre_gg1, self_init1, post1)

        nc.gpsimd.collective_compute(
            "AllGather", mybir.AluOpType.bypass,
            replica_groups=[list(range(NCORES))],
            ins=[t2_shard], outs=[g2_full])

        # ---- layer 2 ----
        def pre_gg2(ggi):
            c0, w = ggc0[ggi], ggs[ggi]
            dv = sbx.tile([F2, GGW], dt.float32, tag="dvgg")
            nc.sync.dma_start(dv[:, :w], dinvr[:, c0:c0 + w])
            return {"dv": dv}

        def self_init2(ggi, pg, w, agg, aux):
            c0 = ggc0[ggi] + pg * 512
            nc.tensor.matmul(agg[:F2, :w], lhsT=w2_sb[:],
                             rhs=h1Ts[:, c0:c0 + w],
                             start=True, stop=False)

        def post2(ggi, pg, lo, w, agg, aux):
            ot = sbe.tile([F2, 512], dt.float32, tag="outT")
            nc.vector.tensor_tensor(ot[:, :w], agg[:F2, :w],
                                    aux["dv"][:, pg * 512:pg * 512 + w],
                                    mybir.AluOpType.mult)
            if with_b2:
                nc.vector.tensor_scalar(ot[:, :w], ot[:, :w], b2_sb[:],
                                        None, mybir.AluOpType.add)
            nc.scalar.dma_start(outT[:, lo:lo + w], ot[:, :w])

        edge_phase(2, sch2, idx2, dstv2, F2, sbg2, max_ilen2, max_ch2,
                   pre_gg2, self_init2, post2)

    nc.compile()
    globals()['_last_nc'] = nc
    return _make_runner(nc, NCORES)


_CACHE = {}


def kernel(x, edge_index, W1, b1, W2, b2):
    x = np.asarray(x, np.float32)
    edge_index = np.asarray(edge_index)
    W1 = np.asarray(W1, np.float32)
    b1 = np.asarray(b1, np.float32)
    W2 = np.asarray(W2, np.float32)
    b2 = np.asarray(b2, np.float32)
    assert not np.any(b1), "b1 != 0 not supported by this kernel"

    src = edge_index[0].astype(np.int64)
    dst = edge_index[1].astype(np.int64)
    deg = (np.bincount(dst, minlength=N_REAL) + 1).astype(np.float64)
    dinv = (1.0 / np.sqrt(deg)).astype(np.float32)

    perm = _balanced_perm(np.bincount(dst, minlength=N_REAL))
    new_src = perm[src]
    new_dst = perm[dst]

    dinv_new = np.zeros(N_PAD, np.float32)
    dinv_new[perm[:N_REAL]] = dinv

    pr1 = new_src // 2
    grp1 = (pr1 // SEGP) * 2 + (new_src % 2)
    row1 = pr1 % SEGP
    grp2 = new_src % 4
    row2 = new_src // 4
    sch1 = _build_schedule(new_src, new_dst, grp1, row1)
    sch2 = _build_schedule(new_src, new_dst, grp2, row2)

    with_b2 = bool(np.any(b2))
    ncol = max(sch1["ncol"], sch2["ncol"])

    ckey = (with_b2, ncol,
            tuple((c["k"],
                   tuple((lo, tuple(tuple(pc) for pc in pieces))
                         for (lo, pieces) in c["windows"]))
                  for c in sch1["calls"] + sch2["calls"]))
    if ckey not in _CACHE:
        _CACHE[ckey] = _compile_and_make_runner(sch1, sch2, ncol, with_b2)
    run = _CACHE[ckey]
    globals()['_last_runner'] = run

    x_new = np.zeros((N_PAD, IN_C), np.float32)
    x_new[perm[:N_REAL]] = x
    xs = dinv_new[:, None] * x_new
    colid_np = np.broadcast_to(np.arange(ncol, dtype=np.float32), (P, ncol))

    w1_bf = _bf16(W1)
    w2_bf = _bf16(W2)
    colid_bf = _bf16(colid_np)

    in_maps = []
    for c in range(NCORES):
        lo, hi = c * NSHARD, (c + 1) * NSHARD
        dv = dinv_new[lo:hi]
        m = {
            "xT": _bf16(xs[lo:hi].T.copy()),
            "w1": w1_bf,
            "w2": w2_bf,
            "colid": colid_bf,
            "dinv2r": _bf16(np.broadcast_to((dv * dv)[None, :], (F1, NSHARD))),
            "dinvr": np.broadcast_to(dv[None, :], (F2, NSHARD)).copy(),
            "idx1": _wrap_idx(sch1["idx_arr"][c]),
            "idx2": _wrap_idx(sch2["idx_arr"][c]),
            "dstv1": _bf16(sch1["dstv"][c]),
            "dstv2": _bf16(sch2["dstv"][c]),
            "cnt1": np.maximum(sch1["cnt"][c], 1).reshape(1, -1),
            "cnt2": np.maximum(sch2["cnt"][c], 1).reshape(1, -1),
        }
        if with_b2:
            m["b2c"] = b2.reshape(F2, 1).astype(np.float32)
        in_maps.append(m)

    globals()['_last_in_maps'] = in_maps
    results, _times = run(in_maps, n_iters=0)
    out_new = np.concatenate(
        [results[c]["outT"].T for c in range(NCORES)], axis=0)
    return out_new[perm[:N_REAL]].astype(np.float32)


# revision 15
# speedup vs baseline: 1.0309x; 1.0309x over previous
"""2-layer GCN (PyG GCNConv semantics) on 8 Trainium2 NeuronCores.

Strategy (self-contained; shapes hardcoded for the nn_GCNEncoder problem):
  - Nodes relabeled (degree-balanced) and partitioned across 8 cores
    (12544 padded nodes each = 98 blocks of 128).
  - Layer algebra:
      table1 = dinv * (x @ W1)                  [per-shard matmul + AllGather]
      s1     = segsum(table1[src]) + self       -> h1s = dinv^2 * relu(s1)
      table2 = h1s @ W2  (W2 folded into the table -> 32-wide L2 messages)
      out    = dinv * (segsum(table2[src]) + self) + b2
  - Edge aggregation: edges sorted by (core, gather-group of 16 dst blocks,
    source region, dst slot). Chunks of 128 edges are aligned to slot-
    stratified windows (cross-core max count <= 128 per window, width <= 32
    slots). Each chunk: SWDGE row-gather (dense, no skips; pads duplicate a
    real row) then ONE TensorE matmul: stationary = gathered messages
    [128 edges, F], moving = narrow one-hot [128, w<=32] built on DVE by
    is_equal(colid, dst-lo), accumulated into a transposed PSUM bank
    [F, 512 slots] per 4-block group (started by a self-loop matmul
    W.T @ featT which also initializes the zero region).
  - The transposed PSUM layout makes the W2 fold and the L2 self-init plain
    matmuls (no transposes anywhere); final output is written transposed
    [32, nodes] and re-transposed on host.
"""
import numpy as np
from contextlib import ExitStack

N_REAL = 100000
N_PAD = 100352            # 8 * 98 * 128
NCORES = 8
NSHARD = N_PAD // NCORES  # 12544
NBLOCKS = NSHARD // 128   # 98
P = 128
F1 = 64                   # hidden width (W1 out)
F2 = 32                   # output width (W2 out)
IN_C = 128
GG_SIZES = [6, 12, 16, 16, 16, 16, 10, 4, 2]   # blocks per gather group
assert sum(GG_SIZES) == NBLOCKS
NGG = len(GG_SIZES)
GG_B0 = np.concatenate([[0], np.cumsum(GG_SIZES)]).astype(np.int64)  # block starts
G_GG = max(GG_SIZES)
GGW = G_GG * P            # max slots per gather group (2048)
NGRP = 4                  # source regions per layer
NCOL = 32                 # one-hot window width (host schedule asserts)
CB = 64                   # chunks per DVE one-hot build batch
NPAIR = N_PAD // 2        # 50176 L1 pair rows
SEGP = NPAIR // 2         # 25088 rows per L1 segment
NQUAD = N_PAD // 4        # 25088 L2 quad rows


def _bf16(a):
    import ml_dtypes
    return np.asarray(a, np.float32).astype(ml_dtypes.bfloat16)


def _balanced_perm(deg):
    """Assign nodes to 784 blocks of 128 balancing per-block degree sums.
    Returns perm: orig_id -> new_id (new_id = block*128 + slot)."""
    import heapq
    nblocks_g = N_PAD // P
    order = np.argsort(-deg, kind="stable")
    blocks = np.empty(N_REAL, np.int64)
    heap = [(0, b) for b in range(nblocks_g)]
    heapq.heapify(heap)
    fill = np.zeros(nblocks_g, np.int64)
    deg_sorted = deg[order]
    for i in range(N_REAL):
        load, b = heapq.heappop(heap)
        blocks[order[i]] = b
        fill[b] += 1
        if fill[b] < P:
            heapq.heappush(heap, (load + int(deg_sorted[i]), b))
    o2 = np.argsort(blocks, kind="stable")
    counts = np.bincount(blocks, minlength=nblocks_g)
    starts = np.concatenate([[0], np.cumsum(counts)[:-1]])
    slots = np.arange(N_REAL) - starts[blocks[o2]]
    new_ids = blocks[o2] * P + slots
    perm_real = np.empty(N_REAL, np.int64)
    perm_real[o2] = new_ids
    used = np.zeros(N_PAD, bool)
    used[perm_real] = True
    return np.concatenate([perm_real, np.flatnonzero(~used)])


def _build_schedule(new_src, new_dst, grp, rowidx):
    """Exact-128-edge chunks with cross-core union windows; -1 tail padding
    (trimmed free by the SWDGE ucode). Windows are split at PSUM bank
    boundaries at emission time."""
    core = new_dst // NSHARD
    block = (new_dst % NSHARD) // P
    gg = np.searchsorted(GG_B0, block, side="right") - 1
    slot = (new_dst % NSHARD) - GG_B0[gg] * P

    ncalls = NGG * NGRP
    call_of_edge = gg * NGRP + grp
    okey = (core * ncalls + call_of_edge) * (GGW + P) + slot
    eorder = np.argsort(okey, kind="stable")
    s_idx = rowidx[eorder].astype(np.int64)
    s_slot = slot[eorder]
    ckey = (core * ncalls + call_of_edge)[eorder]
    cnt = np.bincount(ckey, minlength=NCORES * ncalls).reshape(NCORES, ncalls)
    estart = np.concatenate([[0], np.cumsum(cnt.reshape(-1))[:-1]]).reshape(
        NCORES, ncalls)

    k_call = np.maximum(1, np.ceil(cnt.max(axis=0) / P).astype(np.int64))
    kmax = int(k_call.max())
    caps = k_call * P
    call_base = np.concatenate([[0], np.cumsum(caps)[:-1]])
    nidx = int(caps.sum())
    nchunks = int(k_call.sum())
    chunk_base = np.concatenate([[0], np.cumsum(k_call)[:-1]])

    idx_arr = np.full((NCORES, nidx), -1, np.int16)
    dstv = np.full((NCORES, P, nchunks), -7.0, np.float32)
    win_lo = np.full((ncalls, kmax), 10 ** 9, np.int64)
    win_hi = np.full((ncalls, kmax), -1, np.int64)

    for ci in range(ncalls):
        for c in range(NCORES):
            n = cnt[c, ci]
            b = estart[c, ci]
            if n == 0:
                idx_arr[c, call_base[ci]] = 0
                continue
            sl = s_slot[b:b + n]
            ix = s_idx[b:b + n]
            pos = np.arange(n)
            idx_arr[c, call_base[ci]:call_base[ci] + n] = ix
            wj = pos // P
            np.minimum.at(win_lo[ci], wj, sl)
            np.maximum.at(win_hi[ci], wj, sl)
            dstv[c, pos % P, chunk_base[ci] + wj] = sl

    # relative dst values + window metadata (split at 512-slot banks)
    ncol = 32
    calls = []
    for ci in range(ncalls):
        k = int(k_call[ci])
        meta = []
        for j in range(k):
            lo = int(win_lo[ci, j])
            hi = int(win_hi[ci, j])
            if hi < 0:
                lo, hi = 0, 0
            span = hi - lo + 1
            ncol = max(ncol, span)
            pieces = []
            p0 = lo
            while p0 <= hi:
                pe = min(hi, ((p0 // 512) + 1) * 512 - 1)
                pieces.append([p0, pe - p0 + 1, p0 // 512, False])
                p0 = pe + 1
            meta.append((lo, pieces))
        calls.append({"gg": ci // NGRP, "grp": ci % NGRP, "k": k,
                      "base": int(call_base[ci]),
                      "cbase": int(chunk_base[ci]), "windows": meta})
    ncol = int(np.ceil(ncol / 32) * 32)
    assert ncol <= 256, f"window span {ncol} too wide"

    # dstv relative to window lo
    for ci in range(ncalls):
        k = int(k_call[ci])
        lows = np.array([calls[ci]["windows"][j][0] for j in range(k)],
                        np.float64)
        cb = calls[ci]["cbase"]
        dv = dstv[:, :, cb:cb + k]
        mask = dv >= 0
        dv[mask] = (dv - lows[None, None, :])[mask]

    # stop flag: last piece (consumption order) touching each (gg, pg)
    for ggi in range(NGG):
        last = {}
        for gi in range(NGRP):
            ci = ggi * NGRP + gi
            for j, (lo, pieces) in enumerate(calls[ci]["windows"]):
                for pi, pc in enumerate(pieces):
                    last[pc[2]] = (ci, j, pi)
        for (ci, j, pi) in last.values():
            calls[ci]["windows"][j][1][pi][3] = True

    gg_off, gg_len, gg_cb = [], [], []
    for ggi in range(NGG):
        o = int(call_base[ggi * NGRP])
        e = nidx if ggi == NGG - 1 else int(call_base[(ggi + 1) * NGRP])
        gg_off.append(o)
        gg_len.append(e - o)
        gg_cb.append(int(chunk_base[ggi * NGRP]))
    gg_cb.append(nchunks)

    return {"calls": calls, "idx_arr": idx_arr, "dstv": dstv,
            "gg_off": gg_off, "gg_len": gg_len, "gg_cb": gg_cb,
            "nchunks": nchunks, "nidx": nidx, "kmax": kmax, "ncol": ncol,
            "cnt": cnt.astype(np.int32)}


def _wrap_idx(idx_flat):
    """SWDGE wrapped int16 idx layout: [16, n/16] pattern tiled to 128."""
    n = idx_flat.shape[0]
    cols = idx_flat.astype(np.int16).reshape(n // 16, 16).T
    return np.tile(cols, (8, 1))


def _make_runner(nc, n_cores):
    """Build the bass2jax PJRT executable once; return a callable."""
    import time
    import jax
    from jax.sharding import Mesh, PartitionSpec, NamedSharding
    from jax.experimental.shard_map import shard_map
    import concourse.mybir as mybir
    from concourse.bass2jax import (_bass_exec_p, install_neuronx_cc_hook,
                                    partition_id_tensor)

    install_neuronx_cc_hook()
    partition_name = nc.partition_id_tensor.name if nc.partition_id_tensor else None

    in_names, out_names, out_avals, zero_outs = [], [], [], []
    for alloc in nc.m.functions[0].allocations:
        if not isinstance(alloc, mybir.MemoryLocationSet):
            continue
        name = alloc.memorylocations[0].name
        if alloc.kind == "ExternalInput":
            if name != partition_name:
                in_names.append(name)
        elif alloc.kind == "ExternalOutput":
            out_names.append(name)
            shape = tuple(alloc.tensor_shape)
            dtype = mybir.dt.np(alloc.dtype)
            out_avals.append(jax.core.ShapedArray(shape, dtype))
            zero_outs.append(np.zeros(shape, dtype))
    n_params = len(in_names)
    n_outs = len(out_avals)
    all_in_names = list(in_names) + list(out_names)
    if partition_name is not None:
        all_in_names.append(partition_name)

    def _body(*args):
        operands = list(args)
        if partition_name is not None:
            operands.append(partition_id_tensor())
        outs = _bass_exec_p.bind(
            *operands,
            out_avals=tuple(out_avals),
            in_names=tuple(all_in_names),
            out_names=tuple(out_names),
            lowering_input_output_aliases=(),
            sim_require_finite=False,
            sim_require_nnan=False,
            nc=nc,
        )
        return tuple(outs)

    devices = jax.devices()[:n_cores]
    mesh = Mesh(np.asarray(devices), ("core",))
    in_specs = (PartitionSpec("core"),) * (n_params + n_outs)
    out_specs = (PartitionSpec("core"),) * len(out_names)
    sharded = jax.jit(
        shard_map(_body, mesh=mesh, in_specs=in_specs, out_specs=out_specs,
                  check_rep=False),
        keep_unused=True,
    )

    def run(in_maps, n_iters=0):
        shard = NamedSharding(mesh, PartitionSpec("core"))
        per_core = [[np.asarray(m[name]) for name in in_names] for m in in_maps]
        concat_in = [
            np.concatenate([per_core[c][i] for c in range(n_cores)], axis=0)
            for i in range(n_params)
        ]
        concat_zeros = [
            np.zeros((n_cores * z.shape[0], *z.shape[1:]), z.dtype) for z in zero_outs
        ]
        args = [jax.device_put(a, shard) for a in concat_in + concat_zeros]
        out = sharded(*args)
        jax.block_until_ready(out)
        times = []
        for _ in range(n_iters):
            t0 = time.perf_counter()
            out = sharded(*args)
            jax.block_until_ready(out)
            times.append(time.perf_counter() - t0)
        results = [
            {name: np.asarray(out[i]).reshape(n_cores, *out_avals[i].shape)[c]
             for i, name in enumerate(out_names)}
            for c in range(n_cores)
        ]
        return results, times

    return run


def _compile_and_make_runner(sch1, sch2, ncol, with_b2):
    import concourse.bass as bass
    import concourse.bacc as bacc
    import concourse.mybir as mybir
    import concourse.tile as tile
    from concourse.bass import exact_div

    dt = mybir.dt

    def dma_gather(gp, out_ap, in_ap, idxs_ap, num_idxs, nreg, elem_size,
                   elem_step, q):
        stride_bytes_256 = exact_div(elem_step * dt.size(in_ap.dtype), 256)
        _in_ap = gp.lower_ap_dma(in_ap, for_custom_bir_dma=True)
        return gp.add_instruction(
            mybir.InstDMAGatherAnt(
                name=gp.bass.get_next_instruction_name(),
                ins=[*_in_ap, gp.lower_ap(idxs_ap),
                     gp.lower_val_access(nreg)],
                outs=[gp.lower_ap(out_ap)],
                transpose=False, num_idxs=num_idxs, elem_size=elem_size,
                stride_bytes_256=stride_bytes_256, gen_mode=0,
                single_packet=False, queue_num=q))

    CBL = max(4, 2048 // ncol)
    nc = bacc.Bacc("TRN2", target_bir_lowering=False, debug=False,
                   num_devices=NCORES, num_swdge_queues=4,
                   dynamic_dma_scratch_size=24576)

    xT = nc.dram_tensor("xT", [P, NSHARD], dt.bfloat16, kind="ExternalInput").ap()
    w1 = nc.dram_tensor("w1", [P, F1], dt.bfloat16, kind="ExternalInput").ap()
    w2 = nc.dram_tensor("w2", [F1, F2], dt.bfloat16, kind="ExternalInput").ap()
    colid = nc.dram_tensor("colid", [P, ncol], dt.bfloat16, kind="ExternalInput").ap()
    cnt1 = nc.dram_tensor("cnt1", [1, NGG * NGRP], dt.int32, kind="ExternalInput").ap()
    cnt2 = nc.dram_tensor("cnt2", [1, NGG * NGRP], dt.int32, kind="ExternalInput").ap()
    dinv2r = nc.dram_tensor("dinv2r", [F1, NSHARD], dt.bfloat16, kind="ExternalInput").ap()
    dinvr = nc.dram_tensor("dinvr", [F2, NSHARD], dt.float32, kind="ExternalInput").ap()
    idx1 = nc.dram_tensor("idx1", [P, sch1["nidx"] // 16], dt.int16, kind="ExternalInput").ap()
    idx2 = nc.dram_tensor("idx2", [P, sch2["nidx"] // 16], dt.int16, kind="ExternalInput").ap()
    dstv1 = nc.dram_tensor("dstv1", [P, sch1["nchunks"]], dt.bfloat16, kind="ExternalInput").ap()
    dstv2 = nc.dram_tensor("dstv2", [P, sch2["nchunks"]], dt.bfloat16, kind="ExternalInput").ap()
    if with_b2:
        b2c = nc.dram_tensor("b2c", [F2, 1], dt.float32, kind="ExternalInput").ap()
    outT = nc.dram_tensor("outT", [F2, NSHARD], dt.float32, kind="ExternalOutput").ap()

    t1_shard = nc.dram_tensor("t1_shard", [NPAIR // NCORES, P], dt.bfloat16).ap()
    g1_full = nc.dram_tensor("g1_full", [NPAIR, P], dt.bfloat16, addr_space="Shared").ap()
    t2_shard = nc.dram_tensor("t2_shard", [NQUAD // NCORES, P], dt.bfloat16).ap()
    g2_full = nc.dram_tensor("g2_full", [NQUAD, P], dt.bfloat16, addr_space="Shared").ap()

    ggs = [GG_SIZES[g] * P for g in range(NGG)]      # slots per GG
    ggc0 = [int(GG_B0[g]) * P for g in range(NGG)]   # slot offset of GG
    max_ilen1 = max(sch1["gg_len"]) // 16
    max_ilen2 = max(sch2["gg_len"]) // 16
    max_ch1 = max(sch1["gg_cb"][g + 1] - sch1["gg_cb"][g] for g in range(NGG))
    max_ch2 = max(sch2["gg_cb"][g + 1] - sch2["gg_cb"][g] for g in range(NGG))

    with tile.TileContext(nc) as tc, ExitStack() as ctx:
        sb = ctx.enter_context(tc.tile_pool(name="sb", bufs=1))
        sbx = ctx.enter_context(tc.tile_pool(name="sbx", bufs=2))
        sbg1 = ctx.enter_context(tc.tile_pool(name="sbg1", bufs=7))
        sbg2 = ctx.enter_context(tc.tile_pool(name="sbg2", bufs=7))
        sbi = ctx.enter_context(tc.tile_pool(name="sbi", bufs=2))
        sbs = ctx.enter_context(tc.tile_pool(name="sbs", bufs=4))
        sbe = ctx.enter_context(tc.tile_pool(name="sbe", bufs=2))
        sbst = ctx.enter_context(tc.tile_pool(name="sbst", bufs=2))
        ps_agg = ctx.enter_context(tc.tile_pool(name="psagg", bufs=6,
                                                space="PSUM"))
        ps_s = ctx.enter_context(tc.tile_pool(name="pss", bufs=2, space="PSUM"))

        # ---- resident tiles ----
        w1_sb = sb.tile([P, F1], dt.bfloat16)
        w2_sb = sb.tile([F1, F2], dt.bfloat16)
        colid_sb = sb.tile([P, ncol], dt.bfloat16)
        cnt1_sb = sb.tile([1, NGG * NGRP], dt.int32)
        cnt2_sb = sb.tile([1, NGG * NGRP], dt.int32)
        h1Ts = sb.tile([F1, NSHARD], dt.bfloat16)
        nc.scalar.dma_start(w1_sb[:], w1)
        nc.scalar.dma_start(w2_sb[:], w2)
        nc.scalar.dma_start(colid_sb[:], colid)
        nc.scalar.dma_start(cnt1_sb[:], cnt1)
        nc.scalar.dma_start(cnt2_sb[:], cnt2)
        cnt_reg = nc.gpsimd.alloc_register("gather_cnt")
        if with_b2:
            b2_sb = sb.tile([F2, 1], dt.float32)
            nc.scalar.dma_start(b2_sb[:], b2c)

        for _zi in range(7):
            zt1 = sbg1.tile([P, sch1["kmax"] * F1], dt.bfloat16, tag="gb1",
                            name="gb1")
            nc.vector.memset(zt1[:], 0)
            zt2 = sbg2.tile([P, sch2["kmax"] * F2], dt.bfloat16, tag="gb2",
                            name="gb2")
            nc.vector.memset(zt2[:], 0)

        # ---- phase 1: t1 rows = (dinv*x) @ W1, pair-packed to t1_shard ----
        # node (b, s) -> pair row b*64 + s//2, col (s%2)*64
        t1v = t1_shard.rearrange("(b s2) (two f) -> (s2 two) b f", s2=64, f=F1)
        for ggi in range(NGG):
            nb = ggs[ggi] // P
            c0 = ggc0[ggi]
            xgg = sbx.tile([P, GGW], dt.bfloat16, tag="xgg")
            nc.sync.dma_start(xgg[:, :ggs[ggi]], xT[:, c0:c0 + ggs[ggi]])
            stg = sbst.tile([P, G_GG * F1], dt.bfloat16, tag="p1st")
            for g8 in range(0, nb, 8):
                n8 = min(8, nb - g8)
                h_ps = ps_agg.tile([P, 512], dt.float32, tag="agg",
                                   name="aggps")
                for b in range(g8, g8 + n8):
                    bo = (b - g8) * F1
                    nc.tensor.matmul(h_ps[:, bo:bo + F1],
                                     lhsT=xgg[:, b * P:(b + 1) * P],
                                     rhs=w1_sb[:], start=(b == g8),
                                     stop=(b == g8 + n8 - 1))
                nc.scalar.copy(stg[:, g8 * F1:(g8 + n8) * F1],
                               h_ps[:, :n8 * F1])
            nc.scalar.dma_start(
                t1v[:, int(GG_B0[ggi]):int(GG_B0[ggi]) + nb],
                stg[:, :nb * F1].rearrange("p (b f) -> p b f", f=F1))

        nc.gpsimd.collective_compute(
            "AllGather", mybir.AluOpType.bypass,
            replica_groups=[list(range(NCORES))],
            ins=[t1_shard], outs=[g1_full])

        def edge_phase(layer, sch, idx_dram, dstv_dram, F, gpool, max_ilen,
                       max_ch, pre_gg, self_init, post_pg):
            calls = sch["calls"]
            st = {"tile": None, "b0": -1}

            for ggi in range(NGG):
                npg = (ggs[ggi] + 511) // 512
                aux = pre_gg(ggi)
                islab = sbi.tile([P, max_ilen], dt.int16, tag="isl")
                o, ln = sch["gg_off"][ggi], sch["gg_len"][ggi]
                nc.sync.dma_start(islab[:, :ln // 16],
                                    idx_dram[:, o // 16:(o + ln) // 16])
                cb0, cb1 = sch["gg_cb"][ggi], sch["gg_cb"][ggi + 1]
                dvt = sbi.tile([P, max_ch], dt.bfloat16, tag="dvt")
                nc.sync.dma_start(dvt[:, :cb1 - cb0],
                                    dstv_dram[:, cb0:cb1])
                st["tile"], st["b0"] = None, -1

                def get_s2(lj):
                    b0 = (lj // CBL) * CBL
                    if st["b0"] != b0:
                        cbn = min(CBL, (cb1 - cb0) - b0)
                        s2 = sbs.tile([P, CBL * ncol], dt.bfloat16, tag="s2")
                        nc.vector.tensor_tensor(
                            out=s2[:].rearrange("p (c j) -> p c j", c=CBL)[:, :cbn, :],
                            in0=colid_sb[:, None, :].to_broadcast((P, cbn, ncol)),
                            in1=dvt[:, b0:b0 + cbn, None].to_broadcast(
                                (P, cbn, ncol)),
                            op=mybir.AluOpType.is_equal)
                        st["tile"], st["b0"] = s2, b0
                    return st["tile"], lj - b0

                gbufs = []
                for gi in range(NGRP):
                    ci = ggi * NGRP + gi
                    k = calls[ci]["k"]
                    n = k * P
                    gb = gpool.tile([P, sch["kmax"] * F], dt.bfloat16,
                                    tag=f"gb{layer}", name=f"gb{layer}")
                    nc.gpsimd.reg_load(cnt_reg, (cnt1_sb if layer == 1
                                                 else cnt2_sb)[0:1, ci:ci + 1])
                    co = calls[ci]["base"] - o
                    if layer == 1:
                        seg, par = gi // 2, gi % 2
                        src_ap = g1_full[seg * SEGP:(seg + 1) * SEGP,
                                         par * F1:(par + 1) * F1]
                    else:
                        src_ap = g2_full[:, gi * F2:(gi + 1) * F2]
                    dma_gather(
                        nc.gpsimd,
                        gb[:, :k * F].rearrange("p (c f) -> p c f", f=F),
                        src_ap,
                        islab[:, co // 16:(co + n) // 16],
                        num_idxs=n, nreg=cnt_reg, elem_size=F, elem_step=P,
                        q=gi)
                    gbufs.append(gb)

                aggs = []
                for pg in range(npg):
                    w = min(512, ggs[ggi] - pg * 512)
                    agg = ps_agg.tile([F1, 512], dt.float32, tag="agg",
                                      name="aggps")
                    self_init(ggi, pg, w, agg, aux)
                    aggs.append(agg)

                for gi in range(NGRP):
                    ci = ggi * NGRP + gi
                    cbase = calls[ci]["cbase"] - cb0
                    for j, (lo, pieces) in enumerate(calls[ci]["windows"]):
                        s2, coff = get_s2(cbase + j)
                        for (plo, pw, pg, stop) in pieces:
                            nc.tensor.matmul(
                                aggs[pg][:F if layer == 2 else F1,
                                         plo - pg * 512:plo - pg * 512 + pw],
                                lhsT=gbufs[gi][:, j * F:(j + 1) * F],
                                rhs=s2[:, coff * ncol + (plo - lo):
                                       coff * ncol + (plo - lo) + pw],
                                start=False, stop=stop)

                for pg in range(npg):
                    w = min(512, ggs[ggi] - pg * 512)
                    post_pg(ggi, pg, ggc0[ggi] + pg * 512, w, aggs[pg], aux)

        # ---- layer 1 ----
        def pre_gg1(ggi):
            c0, w = ggc0[ggi], ggs[ggi]
            xgg = sbx.tile([P, GGW], dt.bfloat16, tag="xgg", name="xgg")
            nc.sync.dma_start(xgg[:, :w], xT[:, c0:c0 + w])
            d2 = sbx.tile([F1, GGW], dt.bfloat16, tag="d2gg", name="d2gg")
            nc.sync.dma_start(d2[:, :w], dinv2r[:, c0:c0 + w])
            t2st = sbst.tile([P, G_GG * F2], dt.bfloat16, tag="t2st",
                             name="t2st")
            return {"xgg": xgg, "d2": d2, "t2st": t2st}

        def self_init1(ggi, pg, w, agg, aux):
            nc.tensor.matmul(agg[:, :w], lhsT=w1_sb[:],
                             rhs=aux["xgg"][:, pg * 512:pg * 512 + w],
                             start=True, stop=False)

        def post1(ggi, pg, lo, w, agg, aux):
            tr = sbe.tile([F1, 512], dt.bfloat16, tag="trelu")
            nc.scalar.activation(tr[:, :w], agg[:, :w],
                                 mybir.ActivationFunctionType.Relu)
            nc.vector.tensor_tensor(
                h1Ts[:, lo:lo + w], tr[:, :w],
                aux["d2"][:, pg * 512:pg * 512 + w], mybir.AluOpType.mult)
            for bl in range(w // P):
                b = lo // P + bl
                t2_ps = ps_s.tile([P, F2], dt.float32, tag="sps")
                nc.tensor.matmul(t2_ps[:], lhsT=h1Ts[:, b * P:(b + 1) * P],
                                 rhs=w2_sb[:], start=True, stop=True)
                nc.scalar.copy(
                    aux["t2st"][:, (pg * 4 + bl) * F2:(pg * 4 + bl + 1) * F2],
                    t2_ps[:])
            if lo + w == (ggc0[ggi] + ggs[ggi]):
                # quad-packed write: node (b,s) -> quad row b*32+s//4, col (s%4)*32
                nb = ggs[ggi] // P
                t2v = t2_shard.rearrange("(b s4) (q f) -> (s4 q) b f",
                                         s4=32, f=F2)
                nc.scalar.dma_start(
                    t2v[:, int(GG_B0[ggi]):int(GG_B0[ggi]) + nb],
                    aux["t2st"][:, :nb * F2].rearrange("p (b f) -> p b f", f=F2))

        edge_phase(1, sch1, idx1, dstv1, F1, sbg1, max_ilen1, max_ch1,
                   pre_gg1, self_init1, post1)

        nc.gpsimd.collective_compute(
            "AllGather", mybir.AluOpType.bypass,
            replica_groups=[list(range(NCORES))],
            ins=[t2_shard], outs=[g2_full])

        # ---- layer 2 ----
        def pre_gg2(ggi):
            c0, w = ggc0[ggi], ggs[ggi]
            dv = sbx.tile([F2, GGW], dt.float32, tag="dvgg")
            nc.sync.dma_start(dv[:, :w], dinvr[:, c0:c0 + w])
            return {"dv": dv}

        def self_init2(ggi, pg, w, agg, aux):
            c0 = ggc0[ggi] + pg * 512
            nc.tensor.matmul(agg[:F2, :w], lhsT=w2_sb[:],
                             rhs=h1Ts[:, c0:c0 + w],
                             start=True, stop=False)

        def post2(ggi, pg, lo, w, agg, aux):
            ot = sbe.tile([F2, 512], dt.float32, tag="outT")
            nc.vector.tensor_tensor(ot[:, :w], agg[:F2, :w],
                                    aux["dv"][:, pg * 512:pg * 512 + w],
                                    mybir.AluOpType.mult)
            if with_b2:
                nc.vector.tensor_scalar(ot[:, :w], ot[:, :w], b2_sb[:],
                                        None, mybir.AluOpType.add)
            nc.scalar.dma_start(outT[:, lo:lo + w], ot[:, :w])

        edge_phase(2, sch2, idx2, dstv2, F2, sbg2, max_ilen2, max_ch2,
                   pre_gg2, self_init2, post2)

    nc.compile()
    globals()['_last_nc'] = nc
    return _make_runner(nc, NCORES)


_CACHE = {}


def kernel(x, edge_index, W1, b1, W2, b2):
    x = np.asarray(x, np.float32)
    edge_index = np.asarray(edge_index)
    W1 = np.asarray(W1, np.float32)
    b1 = np.asarray(b1, np.float32)
    W2 = np.asarray(W2, np.float32)
    b2 = np.asarray(b2, np.float32)
    assert not np.any(b1), "b1 != 0 not supported by this kernel"

    src = edge_index[0].astype(np.int64)
    dst = edge_index[1].astype(np.int64)
    deg = (np.bincount(dst, minlength=N_REAL) + 1).astype(np.float64)
    dinv = (1.0 / np.sqrt(deg)).astype(np.float32)

    perm = _balanced_perm(np.bincount(dst, minlength=N_REAL))
    new_src = perm[src]
    new_dst = perm[dst]

    dinv_new = np.zeros(N_PAD, np.float32)
    dinv_new[perm[:N_REAL]] = dinv

    pr1 = new_src // 2
    grp1 = (pr1 // SEGP) * 2 + (new_src % 2)
    row1 = pr1 % SEGP
    grp2 = new_src % 4
    row2 = new_src // 4
    sch1 = _build_schedule(new_src, new_dst, grp1, row1)
    sch2 = _build_schedule(new_src, new_dst, grp2, row2)

    with_b2 = bool(np.any(b2))
    ncol = max(sch1["ncol"], sch2["ncol"])

    ckey = (with_b2, ncol,
            tuple((c["k"],
                   tuple((lo, tuple(tuple(pc) for pc in pieces))
                         for (lo, pieces) in c["windows"]))
                  for c in sch1["calls"] + sch2["calls"]))
    if ckey not in _CACHE:
        _CACHE[ckey] = _compile_and_make_runner(sch1, sch2, ncol, with_b2)
    run = _CACHE[ckey]
    globals()['_last_runner'] = run

    x_new = np.zeros((N_PAD, IN_C), np.float32)
    x_new[perm[:N_REAL]] = x
    xs = dinv_new[:, None] * x_new
    colid_np = np.broadcast_to(np.arange(ncol, dtype=np.float32), (P, ncol))

    w1_bf = _bf16(W1)
    w2_bf = _bf16(W2)
    colid_bf = _bf16(colid_np)

    in_maps = []
    for c in range(NCORES):
        lo, hi = c * NSHARD, (c + 1) * NSHARD
        dv = dinv_new[lo:hi]
        m = {
            "xT": _bf16(xs[lo:hi].T.copy()),
            "w1": w1_bf,
            "w2": w2_bf,
            "colid": colid_bf,
            "dinv2r": _bf16(np.broadcast_to((dv * dv)[None, :], (F1, NSHARD))),
            "dinvr": np.broadcast_to(dv[None, :], (F2, NSHARD)).copy(),
            "idx1": _wrap_idx(sch1["idx_arr"][c]),
            "idx2": _wrap_idx(sch2["idx_arr"][c]),
            "dstv1": _bf16(sch1["dstv"][c]),
            "dstv2": _bf16(sch2["dstv"][c]),
            "cnt1": np.maximum(sch1["cnt"][c], 1).reshape(1, -1),
            "cnt2": np.maximum(sch2["cnt"][c], 1).reshape(1, -1),
        }
        if with_b2:
            m["b2c"] = b2.reshape(F2, 1).astype(np.float32)
        in_maps.append(m)

    globals()['_last_in_maps'] = in_maps
    results, _times = run(in_maps, n_iters=0)
    out_new = np.concatenate(
        [results[c]["outT"].T for c in range(NCORES)], axis=0)
    return out_new[perm[:N_REAL]].astype(np.float32)


# revision 16
# speedup vs baseline: 1.0412x; 1.0099x over previous
"""2-layer GCN (PyG GCNConv semantics) on 8 Trainium2 NeuronCores.

Strategy (self-contained; shapes hardcoded for the nn_GCNEncoder problem):
  - Nodes relabeled (degree-balanced) and partitioned across 8 cores
    (12544 padded nodes each = 98 blocks of 128).
  - Layer algebra:
      table1 = dinv * (x @ W1)                  [per-shard matmul + AllGather]
      s1     = segsum(table1[src]) + self       -> h1s = dinv^2 * relu(s1)
      table2 = h1s @ W2  (W2 folded into the table -> 32-wide L2 messages)
      out    = dinv * (segsum(table2[src]) + self) + b2
  - Edge aggregation: edges sorted by (core, gather-group of 16 dst blocks,
    source region, dst slot). Chunks of 128 edges are aligned to slot-
    stratified windows (cross-core max count <= 128 per window, width <= 32
    slots). Each chunk: SWDGE row-gather (dense, no skips; pads duplicate a
    real row) then ONE TensorE matmul: stationary = gathered messages
    [128 edges, F], moving = narrow one-hot [128, w<=32] built on DVE by
    is_equal(colid, dst-lo), accumulated into a transposed PSUM bank
    [F, 512 slots] per 4-block group (started by a self-loop matmul
    W.T @ featT which also initializes the zero region).
  - The transposed PSUM layout makes the W2 fold and the L2 self-init plain
    matmuls (no transposes anywhere); final output is written transposed
    [32, nodes] and re-transposed on host.
"""
import numpy as np
from contextlib import ExitStack

N_REAL = 100000
N_PAD = 100352            # 8 * 98 * 128
NCORES = 8
NSHARD = N_PAD // NCORES  # 12544
NBLOCKS = NSHARD // 128   # 98
P = 128
F1 = 64                   # hidden width (W1 out)
F2 = 32                   # output width (W2 out)
IN_C = 128
GG_SIZES = [6, 12, 16, 16, 16, 16, 10, 4, 2]   # blocks per gather group
assert sum(GG_SIZES) == NBLOCKS
NGG = len(GG_SIZES)
GG_B0 = np.concatenate([[0], np.cumsum(GG_SIZES)]).astype(np.int64)  # block starts
G_GG = max(GG_SIZES)
GGW = G_GG * P            # max slots per gather group (2048)
NGRP = 4                  # source regions per layer
NCOL = 32                 # one-hot window width (host schedule asserts)
CB = 64                   # chunks per DVE one-hot build batch
NPAIR = N_PAD // 2        # 50176 L1 pair rows
SEGP = NPAIR // 2         # 25088 rows per L1 segment
NQUAD = N_PAD // 4        # 25088 L2 quad rows


def _bf16(a):
    import ml_dtypes
    return np.asarray(a, np.float32).astype(ml_dtypes.bfloat16)


def _balanced_perm(deg):
    """Assign nodes to 784 blocks of 128 balancing per-block degree sums.
    Returns perm: orig_id -> new_id (new_id = block*128 + slot)."""
    import heapq
    nblocks_g = N_PAD // P
    order = np.argsort(-deg, kind="stable")
    blocks = np.empty(N_REAL, np.int64)
    heap = [(0, b) for b in range(nblocks_g)]
    heapq.heapify(heap)
    fill = np.zeros(nblocks_g, np.int64)
    deg_sorted = deg[order]
    for i in range(N_REAL):
        load, b = heapq.heappop(heap)
        blocks[order[i]] = b
        fill[b] += 1
        if fill[b] < P:
            heapq.heappush(heap, (load + int(deg_sorted[i]), b))
    o2 = np.argsort(blocks, kind="stable")
    counts = np.bincount(blocks, minlength=nblocks_g)
    starts = np.concatenate([[0], np.cumsum(counts)[:-1]])
    slots = np.arange(N_REAL) - starts[blocks[o2]]
    new_ids = blocks[o2] * P + slots
    perm_real = np.empty(N_REAL, np.int64)
    perm_real[o2] = new_ids
    used = np.zeros(N_PAD, bool)
    used[perm_real] = True
    return np.concatenate([perm_real, np.flatnonzero(~used)])


def _build_schedule(new_src, new_dst, grp, rowidx):
    """Exact-128-edge chunks with cross-core union windows; -1 tail padding
    (trimmed free by the SWDGE ucode). Windows are split at PSUM bank
    boundaries at emission time."""
    core = new_dst // NSHARD
    block = (new_dst % NSHARD) // P
    gg = np.searchsorted(GG_B0, block, side="right") - 1
    slot = (new_dst % NSHARD) - GG_B0[gg] * P

    ncalls = NGG * NGRP
    call_of_edge = gg * NGRP + grp
    okey = (core * ncalls + call_of_edge) * (GGW + P) + slot
    eorder = np.argsort(okey, kind="stable")
    s_idx = rowidx[eorder].astype(np.int64)
    s_slot = slot[eorder]
    ckey = (core * ncalls + call_of_edge)[eorder]
    cnt = np.bincount(ckey, minlength=NCORES * ncalls).reshape(NCORES, ncalls)
    estart = np.concatenate([[0], np.cumsum(cnt.reshape(-1))[:-1]]).reshape(
        NCORES, ncalls)

    k_call = np.maximum(1, np.ceil(cnt.max(axis=0) / P).astype(np.int64))
    kmax = int(k_call.max())
    caps = k_call * P
    call_base = np.concatenate([[0], np.cumsum(caps)[:-1]])
    nidx = int(caps.sum())
    nchunks = int(k_call.sum())
    chunk_base = np.concatenate([[0], np.cumsum(k_call)[:-1]])

    idx_arr = np.full((NCORES, nidx), -1, np.int16)
    dstv = np.full((NCORES, P, nchunks), -7.0, np.float32)
    win_lo = np.full((ncalls, kmax), 10 ** 9, np.int64)
    win_hi = np.full((ncalls, kmax), -1, np.int64)

    for ci in range(ncalls):
        for c in range(NCORES):
            n = cnt[c, ci]
            b = estart[c, ci]
            if n == 0:
                idx_arr[c, call_base[ci]] = 0
                continue
            sl = s_slot[b:b + n]
            ix = s_idx[b:b + n]
            pos = np.arange(n)
            idx_arr[c, call_base[ci]:call_base[ci] + n] = ix
            wj = pos // P
            np.minimum.at(win_lo[ci], wj, sl)
            np.maximum.at(win_hi[ci], wj, sl)
            dstv[c, pos % P, chunk_base[ci] + wj] = sl

    # relative dst values + window metadata (split at 512-slot banks)
    ncol = 32
    calls = []
    for ci in range(ncalls):
        k = int(k_call[ci])
        meta = []
        for j in range(k):
            lo = int(win_lo[ci, j])
            hi = int(win_hi[ci, j])
            if hi < 0:
                lo, hi = 0, 0
            span = hi - lo + 1
            ncol = max(ncol, span)
            pieces = []
            p0 = lo
            while p0 <= hi:
                pe = min(hi, ((p0 // 512) + 1) * 512 - 1)
                pieces.append([p0, pe - p0 + 1, p0 // 512, False])
                p0 = pe + 1
            meta.append((lo, pieces))
        calls.append({"gg": ci // NGRP, "grp": ci % NGRP, "k": k,
                      "base": int(call_base[ci]),
                      "cbase": int(chunk_base[ci]), "windows": meta})
    ncol = int(np.ceil(ncol / 32) * 32)
    assert ncol <= 256, f"window span {ncol} too wide"

    # dstv relative to window lo
    for ci in range(ncalls):
        k = int(k_call[ci])
        lows = np.array([calls[ci]["windows"][j][0] for j in range(k)],
                        np.float64)
        cb = calls[ci]["cbase"]
        dv = dstv[:, :, cb:cb + k]
        mask = dv >= 0
        dv[mask] = (dv - lows[None, None, :])[mask]

    # stop flag: last piece (consumption order) touching each (gg, pg)
    for ggi in range(NGG):
        last = {}
        for gi in range(NGRP):
            ci = ggi * NGRP + gi
            for j, (lo, pieces) in enumerate(calls[ci]["windows"]):
                for pi, pc in enumerate(pieces):
                    last[pc[2]] = (ci, j, pi)
        for (ci, j, pi) in last.values():
            calls[ci]["windows"][j][1][pi][3] = True

    gg_off, gg_len, gg_cb = [], [], []
    for ggi in range(NGG):
        o = int(call_base[ggi * NGRP])
        e = nidx if ggi == NGG - 1 else int(call_base[(ggi + 1) * NGRP])
        gg_off.append(o)
        gg_len.append(e - o)
        gg_cb.append(int(chunk_base[ggi * NGRP]))
    gg_cb.append(nchunks)

    return {"calls": calls, "idx_arr": idx_arr, "dstv": dstv,
            "gg_off": gg_off, "gg_len": gg_len, "gg_cb": gg_cb,
            "nchunks": nchunks, "nidx": nidx, "kmax": kmax, "ncol": ncol,
            "cnt": cnt.astype(np.int32)}


def _wrap_idx(idx_flat):
    """SWDGE wrapped int16 idx layout: [16, n/16] pattern tiled to 128."""
    n = idx_flat.shape[0]
    cols = idx_flat.astype(np.int16).reshape(n // 16, 16).T
    return np.tile(cols, (8, 1))


def _make_runner(nc, n_cores):
    """Build the bass2jax PJRT executable once; return a callable."""
    import time
    import jax
    from jax.sharding import Mesh, PartitionSpec, NamedSharding
    from jax.experimental.shard_map import shard_map
    import concourse.mybir as mybir
    from concourse.bass2jax import (_bass_exec_p, install_neuronx_cc_hook,
                                    partition_id_tensor)

    install_neuronx_cc_hook()
    partition_name = nc.partition_id_tensor.name if nc.partition_id_tensor else None

    in_names, out_names, out_avals, zero_outs = [], [], [], []
    for alloc in nc.m.functions[0].allocations:
        if not isinstance(alloc, mybir.MemoryLocationSet):
            continue
        name = alloc.memorylocations[0].name
        if alloc.kind == "ExternalInput":
            if name != partition_name:
                in_names.append(name)
        elif alloc.kind == "ExternalOutput":
            out_names.append(name)
            shape = tuple(alloc.tensor_shape)
            dtype = mybir.dt.np(alloc.dtype)
            out_avals.append(jax.core.ShapedArray(shape, dtype))
            zero_outs.append(np.zeros(shape, dtype))
    n_params = len(in_names)
    n_outs = len(out_avals)
    all_in_names = list(in_names) + list(out_names)
    if partition_name is not None:
        all_in_names.append(partition_name)

    def _body(*args):
        operands = list(args)
        if partition_name is not None:
            operands.append(partition_id_tensor())
        outs = _bass_exec_p.bind(
            *operands,
            out_avals=tuple(out_avals),
            in_names=tuple(all_in_names),
            out_names=tuple(out_names),
            lowering_input_output_aliases=(),
            sim_require_finite=False,
            sim_require_nnan=False,
            nc=nc,
        )
        return tuple(outs)

    devices = jax.devices()[:n_cores]
    mesh = Mesh(np.asarray(devices), ("core",))
    in_specs = (PartitionSpec("core"),) * (n_params + n_outs)
    out_specs = (PartitionSpec("core"),) * len(out_names)
    sharded = jax.jit(
        shard_map(_body, mesh=mesh, in_specs=in_specs, out_specs=out_specs,
                  check_rep=False),
        keep_unused=True,
    )

    def run(in_maps, n_iters=0):
        shard = NamedSharding(mesh, PartitionSpec("core"))
        per_core = [[np.asarray(m[name]) for name in in_names] for m in in_maps]
        concat_in = [
            np.concatenate([per_core[c][i] for c in range(n_cores)], axis=0)
            for i in range(n_params)
        ]
        concat_zeros = [
            np.zeros((n_cores * z.shape[0], *z.shape[1:]), z.dtype) for z in zero_outs
        ]
        args = [jax.device_put(a, shard) for a in concat_in + concat_zeros]
        out = sharded(*args)
        jax.block_until_ready(out)
        times = []
        for _ in range(n_iters):
            t0 = time.perf_counter()
            out = sharded(*args)
            jax.block_until_ready(out)
            times.append(time.perf_counter() - t0)
        results = [
            {name: np.asarray(out[i]).reshape(n_cores, *out_avals[i].shape)[c]
             for i, name in enumerate(out_names)}
            for c in range(n_cores)
        ]
        return results, times

    return run


def _compile_and_make_runner(sch1, sch2, ncol, with_b2):
    import concourse.bass as bass
    import concourse.bacc as bacc
    import concourse.mybir as mybir
    import concourse.tile as tile
    from concourse.bass import exact_div

    dt = mybir.dt

    def dma_gather(gp, out_ap, in_ap, idxs_ap, num_idxs, nreg, elem_size,
                   elem_step, q):
        stride_bytes_256 = exact_div(elem_step * dt.size(in_ap.dtype), 256)
        _in_ap = gp.lower_ap_dma(in_ap, for_custom_bir_dma=True)
        return gp.add_instruction(
            mybir.InstDMAGatherAnt(
                name=gp.bass.get_next_instruction_name(),
                ins=[*_in_ap, gp.lower_ap(idxs_ap),
                     gp.lower_val_access(nreg)],
                outs=[gp.lower_ap(out_ap)],
                transpose=False, num_idxs=num_idxs, elem_size=elem_size,
                stride_bytes_256=stride_bytes_256, gen_mode=0,
                single_packet=False, queue_num=q))

    CBL = max(4, 2048 // ncol)
    nc = bacc.Bacc("TRN2", target_bir_lowering=False, debug=False,
                   num_devices=NCORES, num_swdge_queues=4,
                   dynamic_dma_scratch_size=24576)

    xT = nc.dram_tensor("xT", [P, NSHARD], dt.bfloat16, kind="ExternalInput").ap()
    w1 = nc.dram_tensor("w1", [P, F1], dt.bfloat16, kind="ExternalInput").ap()
    w2 = nc.dram_tensor("w2", [F1, F2], dt.bfloat16, kind="ExternalInput").ap()
    colid = nc.dram_tensor("colid", [P, ncol], dt.bfloat16, kind="ExternalInput").ap()
    cnt1 = nc.dram_tensor("cnt1", [1, NGG * NGRP], dt.int32, kind="ExternalInput").ap()
    cnt2 = nc.dram_tensor("cnt2", [1, NGG * NGRP], dt.int32, kind="ExternalInput").ap()
    dinv2r = nc.dram_tensor("dinv2r", [F1, NSHARD], dt.bfloat16, kind="ExternalInput").ap()
    dinvr = nc.dram_tensor("dinvr", [F2, NSHARD], dt.float32, kind="ExternalInput").ap()
    idx1 = nc.dram_tensor("idx1", [P, sch1["nidx"] // 16], dt.int16, kind="ExternalInput").ap()
    idx2 = nc.dram_tensor("idx2", [P, sch2["nidx"] // 16], dt.int16, kind="ExternalInput").ap()
    dstv1 = nc.dram_tensor("dstv1", [P, sch1["nchunks"]], dt.bfloat16, kind="ExternalInput").ap()
    dstv2 = nc.dram_tensor("dstv2", [P, sch2["nchunks"]], dt.bfloat16, kind="ExternalInput").ap()
    if with_b2:
        b2c = nc.dram_tensor("b2c", [F2, 1], dt.float32, kind="ExternalInput").ap()
    outT = nc.dram_tensor("outT", [F2, NSHARD], dt.float32, kind="ExternalOutput").ap()

    t1_shard = nc.dram_tensor("t1_shard", [NPAIR // NCORES, P], dt.bfloat16).ap()
    g1_full = nc.dram_tensor("g1_full", [NPAIR, P], dt.bfloat16, addr_space="Shared").ap()
    t2_shard = nc.dram_tensor("t2_shard", [NQUAD // NCORES, P], dt.bfloat16).ap()
    g2_full = nc.dram_tensor("g2_full", [NQUAD, P], dt.bfloat16, addr_space="Shared").ap()

    ggs = [GG_SIZES[g] * P for g in range(NGG)]      # slots per GG
    ggc0 = [int(GG_B0[g]) * P for g in range(NGG)]   # slot offset of GG
    max_ilen1 = max(sch1["gg_len"]) // 16
    max_ilen2 = max(sch2["gg_len"]) // 16
    max_ch1 = max(sch1["gg_cb"][g + 1] - sch1["gg_cb"][g] for g in range(NGG))
    max_ch2 = max(sch2["gg_cb"][g + 1] - sch2["gg_cb"][g] for g in range(NGG))

    with tile.TileContext(nc) as tc, ExitStack() as ctx:
        sb = ctx.enter_context(tc.tile_pool(name="sb", bufs=1))
        sbx = ctx.enter_context(tc.tile_pool(name="sbx", bufs=2))
        sbg1 = ctx.enter_context(tc.tile_pool(name="sbg1", bufs=7))
        sbg2 = ctx.enter_context(tc.tile_pool(name="sbg2", bufs=7))
        sbi = ctx.enter_context(tc.tile_pool(name="sbi", bufs=2))
        sbs = ctx.enter_context(tc.tile_pool(name="sbs", bufs=4))
        sbe = ctx.enter_context(tc.tile_pool(name="sbe", bufs=2))
        sbst = ctx.enter_context(tc.tile_pool(name="sbst", bufs=2))
        ps_agg = ctx.enter_context(tc.tile_pool(name="psagg", bufs=6,
                                                space="PSUM"))
        ps_s = ctx.enter_context(tc.tile_pool(name="pss", bufs=2, space="PSUM"))

        # ---- resident tiles ----
        w1_sb = sb.tile([P, F1], dt.bfloat16)
        w2_sb = sb.tile([F1, F2], dt.bfloat16)
        colid_sb = sb.tile([P, ncol], dt.bfloat16)
        cnt1_sb = sb.tile([1, NGG * NGRP], dt.int32)
        cnt2_sb = sb.tile([1, NGG * NGRP], dt.int32)
        h1Ts = sb.tile([F1, NSHARD], dt.bfloat16)
        nc.scalar.dma_start(w1_sb[:], w1)
        nc.scalar.dma_start(w2_sb[:], w2)
        nc.scalar.dma_start(colid_sb[:], colid)
        nc.scalar.dma_start(cnt1_sb[:], cnt1)
        nc.scalar.dma_start(cnt2_sb[:], cnt2)
        cnt_reg = nc.gpsimd.alloc_register("gather_cnt")
        if with_b2:
            b2_sb = sb.tile([F2, 1], dt.float32)
            nc.scalar.dma_start(b2_sb[:], b2c)

        for _zi in range(7):
            zt1 = sbg1.tile([P, sch1["kmax"] * F1], dt.bfloat16, tag="gb1",
                            name="gb1")
            nc.vector.memset(zt1[:], 0)
            zt2 = sbg2.tile([P, sch2["kmax"] * F2], dt.bfloat16, tag="gb2",
                            name="gb2")
            nc.vector.memset(zt2[:], 0)

        # ---- phase 1: t1 rows = (dinv*x) @ W1, pair-packed to t1_shard ----
        # node (b, s) -> pair row b*64 + s//2, col (s%2)*64
        t1v = t1_shard.rearrange("(b s2) (two f) -> (s2 two) b f", s2=64, f=F1)
        for ggi in range(NGG):
            nb = ggs[ggi] // P
            c0 = ggc0[ggi]
            xgg = sbx.tile([P, GGW], dt.bfloat16, tag="xgg")
            nc.sync.dma_start(xgg[:, :ggs[ggi]], xT[:, c0:c0 + ggs[ggi]])
            stg = sbst.tile([P, G_GG * F1], dt.bfloat16, tag="p1st")
            for b in range(nb):
                h_ps = ps_s.tile([P, F1], dt.float32, tag="sps")
                nc.tensor.matmul(h_ps[:], lhsT=xgg[:, b * P:(b + 1) * P],
                                 rhs=w1_sb[:], start=True, stop=True)
                nc.scalar.copy(stg[:, b * F1:(b + 1) * F1], h_ps[:])
            nc.scalar.dma_start(
                t1v[:, int(GG_B0[ggi]):int(GG_B0[ggi]) + nb],
                stg[:, :nb * F1].rearrange("p (b f) -> p b f", f=F1))

        nc.gpsimd.collective_compute(
            "AllGather", mybir.AluOpType.bypass,
            replica_groups=[list(range(NCORES))],
            ins=[t1_shard], outs=[g1_full])

        def edge_phase(layer, sch, idx_dram, dstv_dram, F, gpool, max_ilen,
                       max_ch, pre_gg, self_init, post_pg):
            calls = sch["calls"]
            st = {"tile": None, "b0": -1}

            for ggi in range(NGG):
                npg = (ggs[ggi] + 511) // 512
                aux = pre_gg(ggi)
                islab = sbi.tile([P, max_ilen], dt.int16, tag="isl")
                o, ln = sch["gg_off"][ggi], sch["gg_len"][ggi]
                nc.sync.dma_start(islab[:, :ln // 16],
                                    idx_dram[:, o // 16:(o + ln) // 16])
                cb0, cb1 = sch["gg_cb"][ggi], sch["gg_cb"][ggi + 1]
                dvt = sbi.tile([P, max_ch], dt.bfloat16, tag="dvt")
                nc.sync.dma_start(dvt[:, :cb1 - cb0],
                                    dstv_dram[:, cb0:cb1])
                st["tile"], st["b0"] = None, -1

                def get_s2(lj):
                    b0 = (lj // CBL) * CBL
                    if st["b0"] != b0:
                        cbn = min(CBL, (cb1 - cb0) - b0)
                        s2 = sbs.tile([P, CBL * ncol], dt.bfloat16, tag="s2")
                        nc.vector.tensor_tensor(
                            out=s2[:].rearrange("p (c j) -> p c j", c=CBL)[:, :cbn, :],
                            in0=colid_sb[:, None, :].to_broadcast((P, cbn, ncol)),
                            in1=dvt[:, b0:b0 + cbn, None].to_broadcast(
                                (P, cbn, ncol)),
                            op=mybir.AluOpType.is_equal)
                        st["tile"], st["b0"] = s2, b0
                    return st["tile"], lj - b0

                gbufs = []
                for gi in range(NGRP):
                    ci = ggi * NGRP + gi
                    k = calls[ci]["k"]
                    n = k * P
                    gb = gpool.tile([P, sch["kmax"] * F], dt.bfloat16,
                                    tag=f"gb{layer}", name=f"gb{layer}")
                    nc.gpsimd.reg_load(cnt_reg, (cnt1_sb if layer == 1
                                                 else cnt2_sb)[0:1, ci:ci + 1])
                    co = calls[ci]["base"] - o
                    if layer == 1:
                        seg, par = gi // 2, gi % 2
                        src_ap = g1_full[seg * SEGP:(seg + 1) * SEGP,
                                         par * F1:(par + 1) * F1]
                    else:
                        src_ap = g2_full[:, gi * F2:(gi + 1) * F2]
                    dma_gather(
                        nc.gpsimd,
                        gb[:, :k * F].rearrange("p (c f) -> p c f", f=F),
                        src_ap,
                        islab[:, co // 16:(co + n) // 16],
                        num_idxs=n, nreg=cnt_reg, elem_size=F, elem_step=P,
                        q=gi)
                    gbufs.append(gb)

                aggs = []
                for pg in range(npg):
                    w = min(512, ggs[ggi] - pg * 512)
                    agg = ps_agg.tile([F1, 512], dt.float32, tag="agg",
                                      name="aggps")
                    self_init(ggi, pg, w, agg, aux)
                    aggs.append(agg)

                for gi in range(NGRP):
                    ci = ggi * NGRP + gi
                    cbase = calls[ci]["cbase"] - cb0
                    for j, (lo, pieces) in enumerate(calls[ci]["windows"]):
                        s2, coff = get_s2(cbase + j)
                        for (plo, pw, pg, stop) in pieces:
                            nc.tensor.matmul(
                                aggs[pg][:F if layer == 2 else F1,
                                         plo - pg * 512:plo - pg * 512 + pw],
                                lhsT=gbufs[gi][:, j * F:(j + 1) * F],
                                rhs=s2[:, coff * ncol + (plo - lo):
                                       coff * ncol + (plo - lo) + pw],
                                start=False, stop=stop)

                for pg in range(npg):
                    w = min(512, ggs[ggi] - pg * 512)
                    post_pg(ggi, pg, ggc0[ggi] + pg * 512, w, aggs[pg], aux)

        # ---- layer 1 ----
        def pre_gg1(ggi):
            c0, w = ggc0[ggi], ggs[ggi]
            xgg = sbx.tile([P, GGW], dt.bfloat16, tag="xgg", name="xgg")
            nc.sync.dma_start(xgg[:, :w], xT[:, c0:c0 + w])
            d2 = sbx.tile([F1, GGW], dt.bfloat16, tag="d2gg", name="d2gg")
            nc.sync.dma_start(d2[:, :w], dinv2r[:, c0:c0 + w])
            t2st = sbst.tile([P, G_GG * F2], dt.bfloat16, tag="t2st",
                             name="t2st")
            return {"xgg": xgg, "d2": d2, "t2st": t2st}

        def self_init1(ggi, pg, w, agg, aux):
            nc.tensor.matmul(agg[:, :w], lhsT=w1_sb[:],
                             rhs=aux["xgg"][:, pg * 512:pg * 512 + w],
                             start=True, stop=False)

        def post1(ggi, pg, lo, w, agg, aux):
            tr = sbe.tile([F1, 512], dt.bfloat16, tag="trelu")
            nc.scalar.activation(tr[:, :w], agg[:, :w],
                                 mybir.ActivationFunctionType.Relu)
            nc.vector.tensor_tensor(
                h1Ts[:, lo:lo + w], tr[:, :w],
                aux["d2"][:, pg * 512:pg * 512 + w], mybir.AluOpType.mult)
            for bl in range(w // P):
                b = lo // P + bl
                t2_ps = ps_s.tile([P, F2], dt.float32, tag="sps")
                nc.tensor.matmul(t2_ps[:], lhsT=h1Ts[:, b * P:(b + 1) * P],
                                 rhs=w2_sb[:], start=True, stop=True)
                nc.scalar.copy(
                    aux["t2st"][:, (pg * 4 + bl) * F2:(pg * 4 + bl + 1) * F2],
                    t2_ps[:])
            if lo + w == (ggc0[ggi] + ggs[ggi]):
                # quad-packed write: node (b,s) -> quad row b*32+s//4, col (s%4)*32
                nb = ggs[ggi] // P
                t2v = t2_shard.rearrange("(b s4) (q f) -> (s4 q) b f",
                                         s4=32, f=F2)
                nc.scalar.dma_start(
                    t2v[:, int(GG_B0[ggi]):int(GG_B0[ggi]) + nb],
                    aux["t2st"][:, :nb * F2].rearrange("p (b f) -> p b f", f=F2))

        edge_phase(1, sch1, idx1, dstv1, F1, sbg1, max_ilen1, max_ch1,
                   pre_gg1, self_init1, post1)

        nc.gpsimd.collective_compute(
            "AllGather", mybir.AluOpType.bypass,
            replica_groups=[list(range(NCORES))],
            ins=[t2_shard], outs=[g2_full])

        # ---- layer 2 ----
        def pre_gg2(ggi):
            c0, w = ggc0[ggi], ggs[ggi]
            dv = sbx.tile([F2, GGW], dt.float32, tag="dvgg")
            nc.sync.dma_start(dv[:, :w], dinvr[:, c0:c0 + w])
            return {"dv": dv}

        def self_init2(ggi, pg, w, agg, aux):
            c0 = ggc0[ggi] + pg * 512
            nc.tensor.matmul(agg[:F2, :w], lhsT=w2_sb[:],
                             rhs=h1Ts[:, c0:c0 + w],
                             start=True, stop=False)

        def post2(ggi, pg, lo, w, agg, aux):
            ot = sbe.tile([F2, 512], dt.float32, tag="outT")
            nc.vector.tensor_tensor(ot[:, :w], agg[:F2, :w],
                                    aux["dv"][:, pg * 512:pg * 512 + w],
                                    mybir.AluOpType.mult)
            if with_b2:
                nc.vector.tensor_scalar(ot[:, :w], ot[:, :w], b2_sb[:],
                                        None, mybir.AluOpType.add)
            nc.scalar.dma_start(outT[:, lo:lo + w], ot[:, :w])

        edge_phase(2, sch2, idx2, dstv2, F2, sbg2, max_ilen2, max_ch2,
                   pre_gg2, self_init2, post2)

    nc.compile()
    globals()['_last_nc'] = nc
    return _make_runner(nc, NCORES)


_CACHE = {}


def kernel(x, edge_index, W1, b1, W2, b2):
    x = np.asarray(x, np.float32)
    edge_index = np.asarray(edge_index)
    W1 = np.asarray(W1, np.float32)
    b1 = np.asarray(b1, np.float32)
    W2 = np.asarray(W2, np.float32)
    b2 = np.asarray(b2, np.float32)
    assert not np.any(b1), "b1 != 0 not supported by this kernel"

    src = edge_index[0].astype(np.int64)
    dst = edge_index[1].astype(np.int64)
    deg = (np.bincount(dst, minlength=N_REAL) + 1).astype(np.float64)
    dinv = (1.0 / np.sqrt(deg)).astype(np.float32)

    perm = _balanced_perm(np.bincount(dst, minlength=N_REAL))
    new_src = perm[src]
    new_dst = perm[dst]

    dinv_new = np.zeros(N_PAD, np.float32)
    dinv_new[perm[:N_REAL]] = dinv

    pr1 = new_src // 2
    grp1 = (pr1 // SEGP) * 2 + (new_src % 2)
    row1 = pr1 % SEGP
    grp2 = new_src % 4
    row2 = new_src // 4
    sch1 = _build_schedule(new_src, new_dst, grp1, row1)
    sch2 = _build_schedule(new_src, new_dst, grp2, row2)

    with_b2 = bool(np.any(b2))
    ncol = max(sch1["ncol"], sch2["ncol"])

    ckey = (with_b2, ncol,
            tuple((c["k"],
                   tuple((lo, tuple(tuple(pc) for pc in pieces))
                         for (lo, pieces) in c["windows"]))
                  for c in sch1["calls"] + sch2["calls"]))
    if ckey not in _CACHE:
        _CACHE[ckey] = _compile_and_make_runner(sch1, sch2, ncol, with_b2)
    run = _CACHE[ckey]
    globals()['_last_runner'] = run

    x_new = np.zeros((N_PAD, IN_C), np.float32)
    x_new[perm[:N_REAL]] = x
    xs = dinv_new[:, None] * x_new
    colid_np = np.broadcast_to(np.arange(ncol, dtype=np.float32), (P, ncol))

    w1_bf = _bf16(W1)
    w2_bf = _bf16(W2)
    colid_bf = _bf16(colid_np)

    in_maps = []
    for c in range(NCORES):
        lo, hi = c * NSHARD, (c + 1) * NSHARD
        dv = dinv_new[lo:hi]
        m = {
            "xT": _bf16(xs[lo:hi].T.copy()),
            "w1": w1_bf,
            "w2": w2_bf,
            "colid": colid_bf,
            "dinv2r": _bf16(np.broadcast_to((dv * dv)[None, :], (F1, NSHARD))),
            "dinvr": np.broadcast_to(dv[None, :], (F2, NSHARD)).copy(),
            "idx1": _wrap_idx(sch1["idx_arr"][c]),
            "idx2": _wrap_idx(sch2["idx_arr"][c]),
            "dstv1": _bf16(sch1["dstv"][c]),
            "dstv2": _bf16(sch2["dstv"][c]),
            "cnt1": np.maximum(sch1["cnt"][c], 1).reshape(1, -1),
            "cnt2": np.maximum(sch2["cnt"][c], 1).reshape(1, -1),
        }
        if with_b2:
            m["b2c"] = b2.reshape(F2, 1).astype(np.float32)
        in_maps.append(m)

    globals()['_last_in_maps'] = in_maps
    results, _times = run(in_maps, n_iters=0)
    out_new = np.concatenate(
        [results[c]["outT"].T for c in range(NCORES)], axis=0)
    return out_new[perm[:N_REAL]].astype(np.float32)
